# revision 61
# baseline (speedup 1.0000x reference)
"""Causal kernel (nn_CausalKernel) for 8x TRN2 NeuronCores.

Algorithm: sum_n k_n sin(n*r) decomposed via n = a*297 + b:
  sin(n r) = sin_a cos_b + cos_a sin_b with
  sin_b = sin(2pi frac(b * r/2pi)), sin_a = sin(2pi frac(a * 297r/2pi)).
Per-point trig tables are built mode-major ([modes, points]) with a
magic-number round chain (chain1: x*s+MAGIC, chain2: -MAGIC, stt: x*s-m)
producing the signed fraction f in [-0.5, 0.5], then the ScalarE Sin LUT
(valid range [-pi, pi]) maps sin(2pi f) directly and cos(2pi f) =
sin(pi/2 - 2pi|f|) with |f| from ScalarE Abs or a DVE neg+max pair. The
chain ops are distributed across DVE / Pool / ScalarE to balance engine
busy time; the 35937-mode contraction runs on TensorE in bf16.

Pure data parallel: 8 cores x 16384 points; weights replicated.

Host wrapper: the compiled executable (jit of shard_map over the 8-core
mesh) is built once and cached at module level; warm calls upload only the
[N, 2] (t, x^2+y^2+z^2) point data and download the [N] f32 output.
Weights and the (unused, non-donated) output placeholder buffers stay
resident on device between calls.
"""
import sys
sys.path.insert(0, "/opt/trn_rl_repo")

import numpy as np
import ml_dtypes

import concourse.bass as bass
import concourse.mybir as mybir
import concourse.tile as tile

f32 = np.float32
bf16 = ml_dtypes.bfloat16

N_CORES = 8
NPT = 16384            # points per core
NI = 2048              # points per point-tile
NTILES = NPT // NI     # 8
NCH = 512              # matmul moving-dim chunk (one PSUM bank)
NCHUNKS = NI // NCH    # 4

D1, D2 = 297, 121      # n = a*D1 + b
C1 = 99                # D1 contraction chunk rows (3 chunks)
MT = 33                # temporal modes

MAGIC = float(f32(1.5 * 2 ** 23))
INV2PI = float(f32(1.0 / (2 * np.pi)))
TWO_PI_M = float(f32(6.2831845))   # < 2pi so LUT args stay inside [-pi, pi]
PI_HALF = float(f32(np.pi / 2))
DT = mybir.dt


def _build_nc(mass_parameter: float, coupling_strength: float):
    nc = bass.Bass(target_bir_lowering=False)
    AF = mybir.ActivationFunctionType
    OP = mybir.AluOpType

    ts_in = nc.dram_tensor("ts", [NPT, 2], DT.float32, kind="ExternalInput")
    wk_in = nc.dram_tensor("wk", [C1, 3 * D2], DT.bfloat16, kind="ExternalInput")
    sc_in = nc.dram_tensor("sc", [128, 8], DT.float32, kind="ExternalInput")
    tkw_in = nc.dram_tensor("tkw", [MT, 1], DT.bfloat16, kind="ExternalInput")
    out_d = nc.dram_tensor("out", [NPT], DT.float32, kind="ExternalOutput")
    stg_both_d = nc.dram_tensor("stg_both", [2, NPT], DT.float32)
    bpsi_d = nc.dram_tensor("bpsi", [1, NPT], DT.float32)
    bphi_d = nc.dram_tensor("bphi", [1, NPT], DT.float32)
    btab_d = nc.dram_tensor("btab", [1, NPT], DT.float32)

    mp = float(f32(mass_parameter))
    cs = float(f32(coupling_strength))

    with SafeTileContext(nc) as tc:
        with (
            tc.tile_pool(name="const", bufs=1) as cpool,
            tc.tile_pool(name="pm", bufs=1) as pm,          # point-major persistents
            tc.tile_pool(name="bc", bufs=2) as bc,          # broadcast tiles
            tc.tile_pool(name="chain", bufs=3) as ch,       # frac chain scratch
            tc.tile_pool(name="small", bufs=2) as sm,       # t1m/t2m/stg2
            tc.tile_pool(name="tab", bufs=2) as tb,
            tc.tile_pool(name="tab2", bufs=2) as tb2,         # bf16 tables
            tc.tile_pool(name="ps", bufs=2, space="PSUM") as ps,
            tc.tile_pool(name="psr", bufs=1, space="PSUM") as psr,
        ):
            # ---------------- constants ----------------
            sc0 = cpool.tile([128, 8], DT.float32)
            nc.sync.dma_start(sc0[:], sc_in[:])
            sc = cpool.tile([128, 8], DT.float32)
            nc.vector.tensor_copy(out=sc[:], in_=sc0[:])    # absorb DMA sem on DVE
            wk0 = cpool.tile([C1, 3 * D2], DT.bfloat16)
            nc.sync.dma_start(wk0[:], wk_in[:])
            wk = cpool.tile([C1, 3 * D2], DT.bfloat16)
            nc.vector.tensor_copy(out=wk[:], in_=wk0[:])

            tkw0 = cpool.tile([MT, 1], DT.bfloat16)
            nc.sync.dma_start(tkw0[:], tkw_in[:])
            tkw = cpool.tile([MT, 1], DT.bfloat16)
            nc.vector.tensor_copy(out=tkw[:], in_=tkw0[:])

            ones121 = cpool.tile([D2, 1], DT.bfloat16)
            nc.vector.memset(ones121[:], 1.0)
            pi_half_t = cpool.tile([128, 1], DT.float32)
            nc.vector.memset(pi_half_t[:], PI_HALF)
            magic_t = cpool.tile([128, 1], DT.float32)
            nc.vector.memset(magic_t[:], MAGIC)
            nmagic_t = cpool.tile([128, 1], DT.float32)
            nc.vector.memset(nmagic_t[:], -MAGIC)

            # ---------------- stage 0: point-major precompute ----------------
            crd = pm.tile([128, 256], DT.float32)
            nc.sync.dma_start(crd[:], ts_in.rearrange("(p f) c -> p (f c)", p=128))
            crd2 = crd[:].rearrange("p (f c) -> p f c", c=2)

            t_pm = pm.tile([128, 128], DT.float32)
            nc.vector.tensor_copy(out=t_pm[:], in_=crd2[:, :, 0])
            sdsq = pm.tile([128, 128], DT.float32)
            nc.vector.tensor_copy(out=sdsq[:], in_=crd2[:, :, 1])
            r2e = pm.tile([128, 128], DT.float32)
            nc.vector.tensor_scalar_add(out=r2e[:], in0=sdsq[:], scalar1=float(f32(1e-12)))

            # r = sqrt(r2e) with two Newton refinements (HW sqrt LUT is loose)
            r_pm = pm.tile([128, 128], DT.float32)
            nc.scalar.activation(out=r_pm[:], in_=r2e[:], func=AF.Sqrt)
            tmpa = pm.tile([128, 128], DT.float32, tag="w1")
            tmpb = pm.tile([128, 128], DT.float32, tag="w2")
            for _ in range(2):
                nc.vector.reciprocal(out=tmpa[:], in_=r_pm[:])
                nc.vector.tensor_mul(out=tmpb[:], in0=r2e[:], in1=tmpa[:])
                nc.vector.tensor_add(out=tmpb[:], in0=tmpb[:], in1=r_pm[:])
                nc.vector.tensor_scalar_mul(out=r_pm[:], in0=tmpb[:], scalar1=0.5)

            # psi1 = frac(r/2pi), signed
            A0 = pm.tile([128, 128], DT.float32)
            m0 = pm.tile([128, 128], DT.float32)
            psi1 = pm.tile([128, 128], DT.float32)
            nc.vector.tensor_scalar(out=A0[:], in0=r_pm[:], scalar1=INV2PI,
                                    scalar2=MAGIC, op0=OP.mult, op1=OP.add)
            nc.vector.tensor_scalar_add(out=m0[:], in0=A0[:], scalar1=-MAGIC)
            nc.vector.scalar_tensor_tensor(out=psi1[:], in0=r_pm[:], scalar=INV2PI,
                                           in1=m0[:], op0=OP.mult, op1=OP.subtract)

            # phi1 = frac(D1 * r / 2pi) via 12-bit split of r (accuracy for a<=120 amplification)
            SC12 = float(f32(2.0 ** 12))
            c2_64 = np.float64(D1) / (2 * np.pi)
            c2h = float(f32(np.trunc(c2_64 * 2 ** 12) / 2 ** 12))
            c2l = float(f32(c2_64 - np.float64(f32(c2h))))
            c2f = float(f32(c2_64))
            rh = pm.tile([128, 128], DT.float32)
            rl = pm.tile([128, 128], DT.float32)
            nc.vector.tensor_scalar(out=A0[:], in0=r_pm[:], scalar1=SC12,
                                    scalar2=MAGIC, op0=OP.mult, op1=OP.add)
            nc.vector.tensor_scalar_add(out=m0[:], in0=A0[:], scalar1=-MAGIC)
            nc.vector.tensor_scalar_mul(out=rh[:], in0=m0[:], scalar1=float(f32(2.0 ** -12)))
            nc.vector.tensor_sub(out=rl[:], in0=r_pm[:], in1=rh[:])
            # t1 = rh*c2h (exact); f1 = frac(t1)
            t1t = pm.tile([128, 128], DT.float32, tag="w3")
            nc.vector.tensor_scalar(out=A0[:], in0=rh[:], scalar1=c2h,
                                    scalar2=MAGIC, op0=OP.mult, op1=OP.add)
            nc.vector.tensor_scalar_add(out=m0[:], in0=A0[:], scalar1=-MAGIC)
            nc.vector.scalar_tensor_tensor(out=t1t[:], in0=rh[:], scalar=c2h,
                                           in1=m0[:], op0=OP.mult, op1=OP.subtract)
            # rest = rh*c2l + rl*c2 ; ph = f1 + rest ; phi1 = frac(ph)
            nc.vector.tensor_scalar_mul(out=tmpa[:], in0=rl[:], scalar1=c2f)
            nc.vector.scalar_tensor_tensor(out=tmpb[:], in0=rh[:], scalar=c2l,
                                           in1=tmpa[:], op0=OP.mult, op1=OP.add)
            ph_t = pm.tile([128, 128], DT.float32)
            nc.vector.tensor_add(out=ph_t[:], in0=t1t[:], in1=tmpb[:])
            phi1 = pm.tile([128, 128], DT.float32)
            nc.vector.tensor_scalar(out=A0[:], in0=ph_t[:], scalar1=1.0,
                                    scalar2=MAGIC, op0=OP.mult, op1=OP.add)
            nc.vector.tensor_scalar_add(out=m0[:], in0=A0[:], scalar1=-MAGIC)
            nc.vector.tensor_sub(out=phi1[:], in0=ph_t[:], in1=m0[:])

            # |t|, envelope, green, masks, 1/(r+1e-6)
            tabs = pm.tile([128, 128], DT.float32)
            nc.vector.tensor_scalar_mul(out=tabs[:], in0=t_pm[:], scalar1=-1.0)
            nc.vector.tensor_max(out=tabs[:], in0=tabs[:], in1=t_pm[:])

            # bases to DRAM for broadcast-DMA sourcing — stored as soon as they
            # are ready so tile 0's broadcasts overlap the mask/green tail below
            nc.sync.dma_start(bpsi_d[:].rearrange("o (p f) -> (o p) f", p=128), psi1[:])
            nc.sync.dma_start(bphi_d[:].rearrange("o (p f) -> (o p) f", p=128), phi1[:])
            nc.sync.dma_start(btab_d[:].rearrange("o (p f) -> (o p) f", p=128), tabs[:])

            env_pm = pm.tile([128, 128], DT.float32)
            nc.scalar.activation(out=env_pm[:], in_=tabs[:], func=AF.Exp,
                                 scale=float(f32(-0.1)))
            expg = pm.tile([128, 128], DT.float32)
            nc.scalar.activation(out=expg[:], in_=r_pm[:], func=AF.Exp, scale=-mp)
            rinv = pm.tile([128, 128], DT.float32)
            nc.vector.reciprocal(out=rinv[:], in_=r_pm[:])
            green = pm.tile([128, 128], DT.float32)
            nc.vector.tensor_mul(out=green[:], in0=expg[:], in1=rinv[:])
            nc.vector.tensor_scalar_mul(out=green[:], in0=green[:], scalar1=cs)
            rden = pm.tile([128, 128], DT.float32)
            nc.vector.tensor_scalar_add(out=rden[:], in0=r_pm[:], scalar1=float(f32(1e-6)))
            rdinv = pm.tile([128, 128], DT.float32)
            nc.vector.reciprocal(out=rdinv[:], in_=rden[:])

            tsq = pm.tile([128, 128], DT.float32)
            nc.vector.tensor_mul(out=tsq[:], in0=t_pm[:], in1=t_pm[:])
            interval = pm.tile([128, 128], DT.float32)
            nc.vector.tensor_sub(out=interval[:], in0=tsq[:], in1=sdsq[:])
            mg1 = pm.tile([128, 128], DT.float32, tag="w4")
            mg2 = pm.tile([128, 128], DT.float32, tag="w5")
            nc.vector.tensor_scalar(out=mg1[:], in0=interval[:], scalar1=0.0,
                                    scalar2=None, op0=OP.is_gt)
            nc.vector.tensor_scalar(out=mg2[:], in0=t_pm[:], scalar1=0.0,
                                    scalar2=None, op0=OP.is_gt)
            nc.vector.tensor_mul(out=mg1[:], in0=mg1[:], in1=mg2[:])
            nc.vector.tensor_mul(out=green[:], in0=green[:], in1=mg1[:])
            mo1 = pm.tile([128, 128], DT.float32, tag="w4")
            mo2 = pm.tile([128, 128], DT.float32, tag="w5")
            nc.vector.tensor_scalar(out=mo1[:], in0=interval[:], scalar1=0.0,
                                    scalar2=None, op0=OP.is_ge)
            nc.vector.tensor_scalar(out=mo2[:], in0=t_pm[:], scalar1=0.0,
                                    scalar2=None, op0=OP.is_ge)
            maskout = pm.tile([128, 128], DT.float32)
            nc.vector.tensor_mul(out=maskout[:], in0=mo1[:], in1=mo2[:])

            # ---------------- per point-tile mode-major pipeline ----------------
            # signed frac f = base*scal - round(base*scal), chain engines
            # picked per group to balance DVE / Pool / ACT busy time.
            def frac_chain(P, base, scal, ch_eng, stt_eng):
                Ac = ch.tile([P, NI], DT.float32, tag="Ac")
                fc_ = ch.tile([P, NI], DT.float32, tag="fc")
                if ch_eng == "act":
                    nc.scalar.activation(out=Ac[:], in_=base[:], func=AF.Identity,
                                         bias=magic_t[:P], scale=scal)
                    nc.scalar.activation(out=Ac[:], in_=Ac[:], func=AF.Identity,
                                         bias=nmagic_t[:P], scale=1.0)
                else:
                    eng = nc.vector if ch_eng == "dve" else nc.gpsimd
                    eng.tensor_scalar(out=Ac[:], in0=base[:], scalar1=scal,
                                      scalar2=MAGIC, op0=OP.mult, op1=OP.add)
                    eng.tensor_scalar_add(out=Ac[:], in0=Ac[:], scalar1=-MAGIC)
                seng = nc.vector if stt_eng == "dve" else nc.gpsimd
                seng.scalar_tensor_tensor(out=fc_[:], in0=base[:], scalar=scal,
                                          in1=Ac[:], op0=OP.mult, op1=OP.subtract)
                return fc_, Ac

            def abs_of(P, fc_, Ac, eng):
                # writes |f| into Ac in place (its chain value is dead)
                if eng == "act":
                    nc.scalar.activation(out=Ac[:], in_=fc_[:], func=AF.Abs)
                    return Ac
                nc.vector.tensor_scalar_mul(out=Ac[:], in0=fc_[:], scalar1=-1.0)
                nc.vector.tensor_max(out=Ac[:], in0=Ac[:], in1=fc_[:])
                return Ac

            D1_CH = ("dve", "dve", "pool")
            D1_ABS = ("dve", "act", "act")

            def emit_front(tt_i):
                """Broadcast DMAs + frac chains + Sin-LUT tables for one tile."""
                pslc = slice(tt_i * NI, (tt_i + 1) * NI)
                b_psi = bc.tile([C1, NI], DT.float32, tag="b_psi")
                b_phi = bc.tile([D2, NI], DT.float32, tag="b_phi")
                b_tab = bc.tile([MT, NI], DT.float32, tag="b_tab")
                nc.sync.dma_start(b_psi[:], bpsi_d[0:1, pslc].to_broadcast((C1, NI)))
                nc.sync.dma_start(b_phi[:], bphi_d[0:1, pslc].to_broadcast((D2, NI)))
                nc.sync.dma_start(b_tab[:], btab_d[0:1, pslc].to_broadcast((MT, NI)))

                # D1 tables: bf16 sin/cos of b*r, b = c*99 + j
                sin1 = tb2.tile([C1, 3 * NI], DT.bfloat16, tag="sin1")
                cos1 = tb2.tile([C1, 3 * NI], DT.bfloat16, tag="cos1")
                D1_STT = ("dve", "dve", "dve")
                for c in range(3):
                    fc_, Ac = frac_chain(C1, b_psi, sc[:C1, c:c + 1], D1_CH[c], D1_STT[c])
                    nc.scalar.activation(out=sin1[:, c * NI:(c + 1) * NI], in_=fc_[:],
                                         func=AF.Sin, scale=TWO_PI_M)
                    u = abs_of(C1, fc_, Ac, D1_ABS[c])
                    nc.scalar.activation(out=cos1[:, c * NI:(c + 1) * NI], in_=u[:],
                                         func=AF.Sin, scale=-TWO_PI_M, bias=pi_half_t[:C1])

                # D2 tables
                sin2 = tb2.tile([D2, NI], DT.bfloat16, tag="sin2")
                cos2 = tb2.tile([D2, NI], DT.bfloat16, tag="cos2")
                f2_, A2 = frac_chain(D2, b_phi, sc[:D2, 3:4], "pool", "dve")
                nc.scalar.activation(out=sin2[:], in_=f2_[:], func=AF.Sin, scale=TWO_PI_M)
                u2 = abs_of(D2, f2_, A2, "act")
                nc.scalar.activation(out=cos2[:], in_=u2[:], func=AF.Sin,
                                     scale=-TWO_PI_M, bias=pi_half_t[:D2])

                # temporal cos table
                cost = tb2.tile([MT, NI], DT.bfloat16, tag="cost")
                # stt must be DVE: Pool has no scalar_tensor_tensor opcode
                f3_, A3 = frac_chain(MT, b_tab, sc[:MT, 4:5], "pool", "dve")
                u3 = abs_of(MT, f3_, A3, "act")
                nc.scalar.activation(out=cost[:], in_=u3[:], func=AF.Sin,
                                     scale=-TWO_PI_M, bias=pi_half_t[:MT])
                return sin1, cos1, sin2, cos2, cost

            def emit_tail(tt_i, tabs_):
                """Contraction matmuls + PSUM drain for one tile's tables."""
                sin1, cos1, sin2, cos2, cost = tabs_
                pslc = slice(tt_i * NI, (tt_i + 1) * NI)
                # matmuls per 512-column chunk; reduced rows accumulate in R
                # (temporal row lives at partition 32: matmul dest base
                # partition must be 0, 32, or 64)
                R = psr.tile([33, NI], DT.float32, tag="red")
                for q in range(NCHUNKS):
                    cs_ = slice(q * NCH, (q + 1) * NCH)
                    u_ps = ps.tile([D2, NCH], DT.float32, tag="u")
                    v_ps = ps.tile([D2, NCH], DT.float32, tag="v")
                    for c in range(3):
                        gcs = slice(c * NI + q * NCH, c * NI + (q + 1) * NCH)
                        nc.tensor.matmul(u_ps[:], wk[:, c * D2:(c + 1) * D2], cos1[:, gcs],
                                         start=(c == 0), stop=(c == 2))
                        nc.tensor.matmul(v_ps[:], wk[:, c * D2:(c + 1) * D2], sin1[:, gcs],
                                         start=(c == 0), stop=(c == 2))
                    t1m = sm.tile([D2, NCH], DT.bfloat16, tag="t1m")
                    t2m = sm.tile([D2, NCH], DT.bfloat16, tag="t2m")
                    nc.vector.tensor_mul(out=t1m[:], in0=sin2[:, cs_], in1=u_ps[:])
                    nc.vector.tensor_mul(out=t2m[:], in0=cos2[:, cs_], in1=v_ps[:])
                    nc.tensor.matmul(R[0:1, cs_], ones121[:], t1m[:], start=True, stop=False)
                    nc.tensor.matmul(R[0:1, cs_], ones121[:], t2m[:], start=False, stop=True)
                    nc.tensor.matmul(R[32:33, cs_], tkw[:], cost[:, cs_], start=True, stop=True)
                # both reduced rows PSUM->SBUF in one partition-parallel copy
                # (cost is column-bound, so copying 33 rows == copying 1)
                stg2 = sm.tile([33, NI], DT.float32, tag="stg2")
                nc.vector.tensor_copy(out=stg2[:], in_=R[:, :])
                # one partition-strided DMA grabs rows 0 and 32 together; with
                # the software-pipelined emission this sits BEHIND the next
                # tile's broadcasts in the SP queue, so it no longer stalls them
                nc.sync.dma_start(stg_both_d[0:2, pslc], stg2[0:33:32, :])

            # Software pipelining: tile i's matmul tail is emitted AFTER tile
            # i+1's frac stage, so in each engine's program order DVE starts
            # the next tile's chains instead of idling through the ACT->PE
            # latency of the current tile's tables.
            prev_tabs = None
            for tt_i in range(NTILES):
                cur_tabs = emit_front(tt_i)
                if prev_tabs is not None:
                    emit_tail(tt_i - 1, prev_tabs)
                prev_tabs = cur_tabs
            emit_tail(NTILES - 1, prev_tabs)

            # ---------------- tail: point-major combine ----------------
            spat_pm = pm.tile([128, 128], DT.float32)
            temp_pm = pm.tile([128, 128], DT.float32)
            nc.sync.dma_start(spat_pm[:], stg_both_d[0:1, :].rearrange("o (p f) -> (o p) f", p=128))
            nc.sync.dma_start(temp_pm[:], stg_both_d[1:2, :].rearrange("o (p f) -> (o p) f", p=128))
            spat2 = pm.tile([128, 128], DT.float32)
            nc.vector.tensor_mul(out=spat2[:], in0=spat_pm[:], in1=rdinv[:])
            temp2 = pm.tile([128, 128], DT.float32)
            nc.vector.tensor_mul(out=temp2[:], in0=temp_pm[:], in1=env_pm[:])
            nc.vector.tensor_mul(out=spat2[:], in0=spat2[:], in1=temp2[:])
            nc.vector.tensor_add(out=spat2[:], in0=spat2[:], in1=green[:])
            outt = pm.tile([128, 128], DT.float32)
            nc.vector.tensor_mul(out=outt[:], in0=spat2[:], in1=maskout[:])
            nc.sync.dma_start(out_d.rearrange("(p f) -> p f", p=128), outt[:])
    return nc


class SafeTileContext(tile.TileContext):
    """TileContext for a walrus build with tight per-instruction sync-wait
    limits (DMAs: 1, compute: 2). Excess waits are moved onto injected
    single-wait NOPs placed immediately before the instruction on the same
    engine, and the exit drain is split the same way."""

    _WAIT_LIMITS = {"InstDMACopy": 1, "InstDrain": 1, "InstMemSet": 1}
    _DEFAULT_WAIT_LIMIT = 1

    def schedule_and_allocate(self):
        ret = super().schedule_and_allocate()
        nc = self.nc
        eng_obj = {
            mybir.EngineType.PE: nc.tensor,
            mybir.EngineType.DVE: nc.vector,
            mybir.EngineType.Activation: nc.scalar,
            mybir.EngineType.Pool: nc.gpsimd,
            mybir.EngineType.SP: nc.sync,
        }
        # pass 1: collect instructions carrying too many waits
        fixes = []
        for bb in nc.main_func.blocks:
            insts = bb.instructions
            for i, ins in enumerate(insts):
                si = ins.sync_info
                waits = list(si.on_wait) if si and si.on_wait else []
                limit = self._WAIT_LIMITS.get(type(ins).__name__,
                                              self._DEFAULT_WAIT_LIMIT)
                if len(waits) > limit:
                    fixes.append((insts, i, ins, waits, limit))
        # pass 2: apply in reverse index order per list
        for insts, i, ins, waits, limit in sorted(fixes, key=lambda f: -f[1]):
            si = ins.sync_info
            ins.sync_info = mybir.SyncInfo(
                on_wait=waits[-limit:], on_update=list(si.on_update or []))
            at = i
            if (type(ins).__name__ == "InstMatmult" and i > 0
                    and type(insts[i - 1]).__name__ == "InstLdweights"):
                at = i - 1
            for j, w in enumerate(waits[:-limit]):
                nb = eng_obj[ins.engine].nop()
                nop_ins = nb.ins
                # relocate from wherever nop() appended it
                for bb2 in nc.main_func.blocks:
                    if bb2.instructions and bb2.instructions[-1] is nop_ins:
                        bb2.instructions.pop()
                        break
                nop_ins.sync_info = mybir.SyncInfo(on_wait=[w], on_update=[])
                insts.insert(at + j, nop_ins)
        return ret

    def _drain_and_barrier(self, tick_clock, wait_clock):
        nc = self.nc
        nop0 = nc.sync.nop()
        wait_clock.add_sem_waits(nop0.ins, tile.ScopedClock({None: tick_clock.global_clock}))
        waits = list(nop0.ins.sync_info.on_wait or []) if nop0.ins.sync_info else []
        if len(waits) > 1:
            upd = nop0.ins.sync_info.on_update or []
            nop0.ins.sync_info = mybir.SyncInfo(on_wait=[waits[0]], on_update=list(upd))
            for w in waits[1:]:
                nk = nc.sync.nop()
                nk.ins.sync_info = mybir.SyncInfo(on_wait=[w], on_update=[])
        nc.sync.drain()
        nc.all_engine_barrier()
        assert self.sems is not None
        popped = nc._tile_sem_poison_stack.pop()
        assert popped is self._sem_poison
        nc.clear_and_free_semaphores(list(self.sems.allocated().values()))
        nc.all_engine_barrier()


def _host_constants(spatial_kernel, temporal_kernel):
    k = np.asarray(spatial_kernel, dtype=f32)
    K = k.reshape(D2, D1)                       # K[a, b] = k[a*D1 + b]
    wk = np.empty((C1, 3 * D2), dtype=bf16)
    for c in range(3):
        wk[:, c * D2:(c + 1) * D2] = K[:, c * C1:(c + 1) * C1].T.astype(bf16)
    sc = np.zeros((128, 8), dtype=f32)
    p = np.arange(128, dtype=f32)
    sc[:, 0] = p
    sc[:, 1] = 99 + p
    sc[:, 2] = 198 + p
    sc[:, 3] = p
    freqs = ((np.arange(MT, dtype=f32) + f32(1.0)) * f32(0.1)).astype(f32)
    sc[:MT, 4] = (freqs * f32(INV2PI)).astype(f32)
    tkw = np.asarray(temporal_kernel, dtype=f32).reshape(MT, 1).astype(bf16)
    return wk, sc, tkw


class _Runner:
    """Compile once; keep the jitted shard_map executable, the device-resident
    output placeholder buffers, and (content-keyed) device-resident weights
    alive across kernel() calls so a warm call is a single PJRT dispatch."""

    def __init__(self, nc):
        import jax
        from jax.sharding import Mesh, PartitionSpec, NamedSharding
        from jax.experimental.shard_map import shard_map
        from concourse import bass2jax

        try:
            jax.config.update("jax_compilation_cache_dir",
                              "/tmp/nn_causal_jax_cache")
            jax.config.update("jax_persistent_cache_min_compile_time_secs", 0.0)
            jax.config.update("jax_persistent_cache_min_entry_size_bytes", -1)
        except Exception:
            pass
        bass2jax.install_neuronx_cc_hook()
        self._jax = jax
        partition_name = (nc.partition_id_tensor.name
                          if nc.partition_id_tensor else None)
        in_names, out_names, out_avals, zero_outs = [], [], [], []
        for alloc in nc.m.functions[0].allocations:
            if not isinstance(alloc, mybir.MemoryLocationSet):
                continue
            name = alloc.memorylocations[0].name
            if alloc.kind == "ExternalInput":
                if name != partition_name:
                    in_names.append(name)
            elif alloc.kind == "ExternalOutput":
                shape = tuple(alloc.tensor_shape)
                dtype = mybir.dt.np(alloc.dtype)
                out_names.append(name)
                out_avals.append(jax.core.ShapedArray(shape, dtype))
                zero_outs.append(np.zeros(shape, dtype))
        self.in_names = list(in_names)
        self.out_names = list(out_names)
        n_params = len(in_names)
        n_outs = len(out_avals)
        in_names_all = in_names + out_names
        if partition_name is not None:
            in_names_all.append(partition_name)

        def _body(*args):
            operands = list(args)
            if partition_name is not None:
                operands.append(bass2jax.partition_id_tensor())
            outs = bass2jax._bass_exec_p.bind(
                *operands,
                out_avals=tuple(out_avals),
                in_names=tuple(in_names_all),
                out_names=tuple(out_names),
                lowering_input_output_aliases=(),
                sim_require_finite=True,
                sim_require_nnan=True,
                nc=nc,
            )
            return tuple(outs)

        devices = jax.devices()[:N_CORES]
        assert len(devices) == N_CORES
        mesh = Mesh(np.asarray(devices), ("core",))
        self._sharding = NamedSharding(mesh, PartitionSpec("core"))
        self._fn = jax.jit(
            shard_map(_body, mesh=mesh,
                      in_specs=(PartitionSpec("core"),) * (n_params + n_outs),
                      out_specs=(PartitionSpec("core"),) * n_outs,
                      check_rep=False),
            keep_unused=True,
        )
        # The kernel fully writes its only output, so the zero placeholder
        # buffers are never read: keep them device-resident, undonated.
        self._zeros_dev = [
            jax.device_put(
                np.zeros((N_CORES * z.shape[0], *z.shape[1:]), z.dtype),
                self._sharding)
            for z in zero_outs
        ]
        self._const_key = None
        self._const_dev = None

    def __call__(self, ts_global, wk, sc, tkw):
        jax = self._jax
        ckey = (wk.tobytes(), sc.tobytes(), tkw.tobytes())
        if self._const_key != ckey:
            self._const_dev = {
                name: jax.device_put(
                    np.concatenate([arr] * N_CORES, axis=0), self._sharding)
                for name, arr in (("wk", wk), ("sc", sc), ("tkw", tkw))
            }
            self._const_key = ckey
        args = []
        for name in self.in_names:
            if name == "ts":
                args.append(ts_global)
            else:
                args.append(self._const_dev[name])
        out_arrs = self._fn(*args, *self._zeros_dev)
        return np.asarray(out_arrs[self.out_names.index("out")])


_RUNNER = None
_RUNNER_KEY = None


def _run_fallback(nc, ts, wk, sc, tkw):
    """Stock path: per-call run_bass_kernel_spmd (no executable caching)."""
    from concourse.bass_utils import run_bass_kernel_spmd
    in_maps = [{"ts": np.ascontiguousarray(ts[c * NPT:(c + 1) * NPT]),
                "wk": wk, "sc": sc, "tkw": tkw} for c in range(N_CORES)]
    res = run_bass_kernel_spmd(nc, in_maps, core_ids=list(range(N_CORES)))
    return np.concatenate([res.results[c]["out"] for c in range(N_CORES)])


def kernel(spacetime_coords, spatial_kernel, temporal_kernel,
           mass_parameter, coupling_strength):
    global _RUNNER, _RUNNER_KEY
    coords = np.asarray(spacetime_coords, dtype=np.float32)
    t = coords[:, 0]
    x = coords[:, 1]
    y = coords[:, 2]
    z = coords[:, 3]
    ts = np.empty((coords.shape[0], 2), dtype=np.float32)
    ts[:, 0] = t
    ts[:, 1] = (x * x + y * y) + z * z
    wk, sc, tkw = _host_constants(spatial_kernel, temporal_kernel)

    key = (float(np.float32(mass_parameter)), float(np.float32(coupling_strength)))
    if _RUNNER is None or _RUNNER_KEY != key:
        nc = _build_nc(*key)
        try:
            runner = _Runner(nc)
            runner(ts, wk, sc, tkw)   # warm the dispatch path during setup
        except Exception:
            return _run_fallback(nc, ts, wk, sc, tkw)
        _RUNNER = runner
        _RUNNER_KEY = key
    return _RUNNER(ts, wk, sc, tkw)


if __name__ == "__main__":
    rng = np.random.default_rng(0)
    ins = {
        "spacetime_coords": (rng.standard_normal((131072, 4)) * 2.0).astype(np.float32),
        "spatial_kernel": (rng.standard_normal(35937) * 0.1).astype(np.float32),
        "temporal_kernel": (rng.standard_normal(33) * 0.1).astype(np.float32),
        "mass_parameter": np.float32(1.0),
        "coupling_strength": np.float32(0.1),
    }
    out = kernel(**ins)
    print("out", out.shape, out.dtype, float(np.abs(out).max()))
    out2 = kernel(**ins)
    print("match:", np.array_equal(out, out2))


# revision 62
# speedup vs baseline: 1.0937x; 1.0937x over previous
"""Causal kernel (nn_CausalKernel) for 8x TRN2 NeuronCores.

Algorithm: sum_n k_n sin(n*r) decomposed via n = a*297 + b:
  sin(n r) = sin_a cos_b + cos_a sin_b with
  sin_b = sin(2pi frac(b * r/2pi)), sin_a = sin(2pi frac(a * 297r/2pi)).
Per-point trig tables are built mode-major ([modes, points]) with a
magic-number round chain (chain1: x*s+MAGIC, chain2: -MAGIC, stt: x*s-m)
producing the signed fraction f in [-0.5, 0.5], then the ScalarE Sin LUT
(valid range [-pi, pi]) maps sin(2pi f) directly and cos(2pi f) =
sin(pi/2 - 2pi|f|) with |f| from ScalarE Abs or a DVE neg+max pair. The
chain ops are distributed across DVE / Pool / ScalarE to balance engine
busy time; the 35937-mode contraction runs on TensorE in bf16.

Pure data parallel: 8 cores x 16384 points; weights replicated.

Host wrapper: the compiled executable (jit of shard_map over the 8-core
mesh) is built once and cached at module level; warm calls upload only the
[N, 2] (t, x^2+y^2+z^2) point data and download the [N] f32 output.
Weights and the (unused, non-donated) output placeholder buffers stay
resident on device between calls.
"""
import sys
sys.path.insert(0, "/opt/trn_rl_repo")

import numpy as np
import ml_dtypes

import concourse.bass as bass
import concourse.mybir as mybir
import concourse.tile as tile

f32 = np.float32
bf16 = ml_dtypes.bfloat16

N_CORES = 8
NPT = 16384            # points per core
NI = 2048              # points per point-tile
NTILES = NPT // NI     # 8
NCH = 512              # matmul moving-dim chunk (one PSUM bank)
NCHUNKS = NI // NCH    # 4

D1, D2 = 297, 121      # n = a*D1 + b
C1 = 99                # D1 contraction chunk rows (3 chunks)
MT = 33                # temporal modes

MAGIC = float(f32(1.5 * 2 ** 23))
INV2PI = float(f32(1.0 / (2 * np.pi)))
TWO_PI_M = float(f32(6.2831845))   # < 2pi so LUT args stay inside [-pi, pi]
PI_HALF = float(f32(np.pi / 2))
DT = mybir.dt


def _build_nc(mass_parameter: float, coupling_strength: float):
    nc = bass.Bass(target_bir_lowering=False)
    AF = mybir.ActivationFunctionType
    OP = mybir.AluOpType

    ts_in = nc.dram_tensor("ts", [NPT, 2], DT.float32, kind="ExternalInput")
    wk_in = nc.dram_tensor("wk", [C1, 3 * D2], DT.bfloat16, kind="ExternalInput")
    sc_in = nc.dram_tensor("sc", [128, 8], DT.float32, kind="ExternalInput")
    tkw_in = nc.dram_tensor("tkw", [MT, 1], DT.bfloat16, kind="ExternalInput")
    out_d = nc.dram_tensor("out", [NPT], DT.float32, kind="ExternalOutput")
    stg_both_d = nc.dram_tensor("stg_both", [2, NPT], DT.float32)
    bpsi_d = nc.dram_tensor("bpsi", [1, NPT], DT.float32)
    bphi_d = nc.dram_tensor("bphi", [1, NPT], DT.float32)
    btab_d = nc.dram_tensor("btab", [1, NPT], DT.float32)

    mp = float(f32(mass_parameter))
    cs = float(f32(coupling_strength))

    with SafeTileContext(nc) as tc:
        with (
            tc.tile_pool(name="const", bufs=1) as cpool,
            tc.tile_pool(name="pm", bufs=1) as pm,          # point-major persistents
            tc.tile_pool(name="bc", bufs=2) as bc,          # broadcast tiles
            tc.tile_pool(name="chain", bufs=3) as ch,       # frac chain scratch
            tc.tile_pool(name="small", bufs=2) as sm,       # t1m/t2m/stg2
            tc.tile_pool(name="tab", bufs=2) as tb,
            tc.tile_pool(name="tab2", bufs=2) as tb2,         # bf16 tables
            tc.tile_pool(name="ps", bufs=2, space="PSUM") as ps,
            tc.tile_pool(name="psr", bufs=1, space="PSUM") as psr,
        ):
            # ---------------- constants ----------------
            sc0 = cpool.tile([128, 8], DT.float32)
            nc.sync.dma_start(sc0[:], sc_in[:])
            sc = cpool.tile([128, 8], DT.float32)
            nc.vector.tensor_copy(out=sc[:], in_=sc0[:])    # absorb DMA sem on DVE
            wk0 = cpool.tile([C1, 3 * D2], DT.bfloat16)
            nc.sync.dma_start(wk0[:], wk_in[:])
            wk = cpool.tile([C1, 3 * D2], DT.bfloat16)
            nc.vector.tensor_copy(out=wk[:], in_=wk0[:])

            tkw0 = cpool.tile([MT, 1], DT.bfloat16)
            nc.sync.dma_start(tkw0[:], tkw_in[:])
            tkw = cpool.tile([MT, 1], DT.bfloat16)
            nc.vector.tensor_copy(out=tkw[:], in_=tkw0[:])

            ones121 = cpool.tile([D2, 1], DT.bfloat16)
            nc.vector.memset(ones121[:], 1.0)
            pi_half_t = cpool.tile([128, 1], DT.float32)
            nc.vector.memset(pi_half_t[:], PI_HALF)
            magic_t = cpool.tile([128, 1], DT.float32)
            nc.vector.memset(magic_t[:], MAGIC)
            nmagic_t = cpool.tile([128, 1], DT.float32)
            nc.vector.memset(nmagic_t[:], -MAGIC)

            # ---------------- stage 0: point-major precompute ----------------
            crd = pm.tile([128, 256], DT.float32)
            nc.sync.dma_start(crd[:], ts_in.rearrange("(p f) c -> p (f c)", p=128))
            crd2 = crd[:].rearrange("p (f c) -> p f c", c=2)

            t_pm = pm.tile([128, 128], DT.float32)
            nc.vector.tensor_copy(out=t_pm[:], in_=crd2[:, :, 0])
            sdsq = pm.tile([128, 128], DT.float32)
            nc.vector.tensor_copy(out=sdsq[:], in_=crd2[:, :, 1])
            r2e = pm.tile([128, 128], DT.float32)
            nc.vector.tensor_scalar_add(out=r2e[:], in0=sdsq[:], scalar1=float(f32(1e-12)))

            # r = sqrt(r2e) with two Newton refinements (HW sqrt LUT is loose)
            r_pm = pm.tile([128, 128], DT.float32)
            nc.scalar.activation(out=r_pm[:], in_=r2e[:], func=AF.Sqrt)
            tmpa = pm.tile([128, 128], DT.float32, tag="w1")
            tmpb = pm.tile([128, 128], DT.float32, tag="w2")
            for _ in range(2):
                nc.vector.reciprocal(out=tmpa[:], in_=r_pm[:])
                nc.vector.tensor_mul(out=tmpb[:], in0=r2e[:], in1=tmpa[:])
                nc.vector.tensor_add(out=tmpb[:], in0=tmpb[:], in1=r_pm[:])
                nc.vector.tensor_scalar_mul(out=r_pm[:], in0=tmpb[:], scalar1=0.5)

            # psi1 = frac(r/2pi), signed
            A0 = pm.tile([128, 128], DT.float32)
            m0 = pm.tile([128, 128], DT.float32)
            psi1 = pm.tile([128, 128], DT.float32)
            nc.vector.tensor_scalar(out=A0[:], in0=r_pm[:], scalar1=INV2PI,
                                    scalar2=MAGIC, op0=OP.mult, op1=OP.add)
            nc.vector.tensor_scalar_add(out=m0[:], in0=A0[:], scalar1=-MAGIC)
            nc.vector.scalar_tensor_tensor(out=psi1[:], in0=r_pm[:], scalar=INV2PI,
                                           in1=m0[:], op0=OP.mult, op1=OP.subtract)

            # phi1 = frac(D1 * r / 2pi) via 12-bit split of r (accuracy for a<=120 amplification)
            SC12 = float(f32(2.0 ** 12))
            c2_64 = np.float64(D1) / (2 * np.pi)
            c2h = float(f32(np.trunc(c2_64 * 2 ** 12) / 2 ** 12))
            c2l = float(f32(c2_64 - np.float64(f32(c2h))))
            c2f = float(f32(c2_64))
            rh = pm.tile([128, 128], DT.float32)
            rl = pm.tile([128, 128], DT.float32)
            nc.vector.tensor_scalar(out=A0[:], in0=r_pm[:], scalar1=SC12,
                                    scalar2=MAGIC, op0=OP.mult, op1=OP.add)
            nc.vector.tensor_scalar_add(out=m0[:], in0=A0[:], scalar1=-MAGIC)
            nc.vector.tensor_scalar_mul(out=rh[:], in0=m0[:], scalar1=float(f32(2.0 ** -12)))
            nc.vector.tensor_sub(out=rl[:], in0=r_pm[:], in1=rh[:])
            # t1 = rh*c2h (exact); f1 = frac(t1)
            t1t = pm.tile([128, 128], DT.float32, tag="w3")
            nc.vector.tensor_scalar(out=A0[:], in0=rh[:], scalar1=c2h,
                                    scalar2=MAGIC, op0=OP.mult, op1=OP.add)
            nc.vector.tensor_scalar_add(out=m0[:], in0=A0[:], scalar1=-MAGIC)
            nc.vector.scalar_tensor_tensor(out=t1t[:], in0=rh[:], scalar=c2h,
                                           in1=m0[:], op0=OP.mult, op1=OP.subtract)
            # rest = rh*c2l + rl*c2 ; ph = f1 + rest ; phi1 = frac(ph)
            nc.vector.tensor_scalar_mul(out=tmpa[:], in0=rl[:], scalar1=c2f)
            nc.vector.scalar_tensor_tensor(out=tmpb[:], in0=rh[:], scalar=c2l,
                                           in1=tmpa[:], op0=OP.mult, op1=OP.add)
            ph_t = pm.tile([128, 128], DT.float32)
            nc.vector.tensor_add(out=ph_t[:], in0=t1t[:], in1=tmpb[:])
            phi1 = pm.tile([128, 128], DT.float32)
            nc.vector.tensor_scalar(out=A0[:], in0=ph_t[:], scalar1=1.0,
                                    scalar2=MAGIC, op0=OP.mult, op1=OP.add)
            nc.vector.tensor_scalar_add(out=m0[:], in0=A0[:], scalar1=-MAGIC)
            nc.vector.tensor_sub(out=phi1[:], in0=ph_t[:], in1=m0[:])

            # |t|, envelope, green, masks, 1/(r+1e-6)
            tabs = pm.tile([128, 128], DT.float32)
            nc.vector.tensor_scalar_mul(out=tabs[:], in0=t_pm[:], scalar1=-1.0)
            nc.vector.tensor_max(out=tabs[:], in0=tabs[:], in1=t_pm[:])

            # bases to DRAM for broadcast-DMA sourcing — stored as soon as they
            # are ready so tile 0's broadcasts overlap the mask/green tail below
            nc.sync.dma_start(bpsi_d[:].rearrange("o (p f) -> (o p) f", p=128), psi1[:])
            nc.sync.dma_start(bphi_d[:].rearrange("o (p f) -> (o p) f", p=128), phi1[:])
            nc.sync.dma_start(btab_d[:].rearrange("o (p f) -> (o p) f", p=128), tabs[:])

            env_pm = pm.tile([128, 128], DT.float32)
            nc.scalar.activation(out=env_pm[:], in_=tabs[:], func=AF.Exp,
                                 scale=float(f32(-0.1)))
            expg = pm.tile([128, 128], DT.float32)
            nc.scalar.activation(out=expg[:], in_=r_pm[:], func=AF.Exp, scale=-mp)
            rinv = pm.tile([128, 128], DT.float32)
            nc.vector.reciprocal(out=rinv[:], in_=r_pm[:])
            green = pm.tile([128, 128], DT.float32)
            nc.vector.tensor_mul(out=green[:], in0=expg[:], in1=rinv[:])
            nc.vector.tensor_scalar_mul(out=green[:], in0=green[:], scalar1=cs)
            rden = pm.tile([128, 128], DT.float32)
            nc.vector.tensor_scalar_add(out=rden[:], in0=r_pm[:], scalar1=float(f32(1e-6)))
            rdinv = pm.tile([128, 128], DT.float32)
            nc.vector.reciprocal(out=rdinv[:], in_=rden[:])

            tsq = pm.tile([128, 128], DT.float32)
            nc.vector.tensor_mul(out=tsq[:], in0=t_pm[:], in1=t_pm[:])
            interval = pm.tile([128, 128], DT.float32)
            nc.vector.tensor_sub(out=interval[:], in0=tsq[:], in1=sdsq[:])
            mg1 = pm.tile([128, 128], DT.float32, tag="w4")
            mg2 = pm.tile([128, 128], DT.float32, tag="w5")
            nc.vector.tensor_scalar(out=mg1[:], in0=interval[:], scalar1=0.0,
                                    scalar2=None, op0=OP.is_gt)
            nc.vector.tensor_scalar(out=mg2[:], in0=t_pm[:], scalar1=0.0,
                                    scalar2=None, op0=OP.is_gt)
            nc.vector.tensor_mul(out=mg1[:], in0=mg1[:], in1=mg2[:])
            nc.vector.tensor_mul(out=green[:], in0=green[:], in1=mg1[:])
            mo1 = pm.tile([128, 128], DT.float32, tag="w4")
            mo2 = pm.tile([128, 128], DT.float32, tag="w5")
            nc.vector.tensor_scalar(out=mo1[:], in0=interval[:], scalar1=0.0,
                                    scalar2=None, op0=OP.is_ge)
            nc.vector.tensor_scalar(out=mo2[:], in0=t_pm[:], scalar1=0.0,
                                    scalar2=None, op0=OP.is_ge)
            maskout = pm.tile([128, 128], DT.float32)
            nc.vector.tensor_mul(out=maskout[:], in0=mo1[:], in1=mo2[:])

            # ---------------- per point-tile mode-major pipeline ----------------
            # signed frac f = base*scal - round(base*scal), chain engines
            # picked per group to balance DVE / Pool / ACT busy time.
            def frac_chain(P, base, scal, ch_eng, stt_eng):
                Ac = ch.tile([P, NI], DT.float32, tag="Ac")
                fc_ = ch.tile([P, NI], DT.float32, tag="fc")
                if ch_eng == "act":
                    nc.scalar.activation(out=Ac[:], in_=base[:], func=AF.Identity,
                                         bias=magic_t[:P], scale=scal)
                    nc.scalar.activation(out=Ac[:], in_=Ac[:], func=AF.Identity,
                                         bias=nmagic_t[:P], scale=1.0)
                else:
                    eng = nc.vector if ch_eng == "dve" else nc.gpsimd
                    eng.tensor_scalar(out=Ac[:], in0=base[:], scalar1=scal,
                                      scalar2=MAGIC, op0=OP.mult, op1=OP.add)
                    eng.tensor_scalar_add(out=Ac[:], in0=Ac[:], scalar1=-MAGIC)
                seng = nc.vector if stt_eng == "dve" else nc.gpsimd
                seng.scalar_tensor_tensor(out=fc_[:], in0=base[:], scalar=scal,
                                          in1=Ac[:], op0=OP.mult, op1=OP.subtract)
                return fc_, Ac

            def abs_of(P, fc_, Ac, eng):
                # writes |f| into Ac in place (its chain value is dead)
                if eng == "act":
                    nc.scalar.activation(out=Ac[:], in_=fc_[:], func=AF.Abs)
                    return Ac
                nc.vector.tensor_scalar_mul(out=Ac[:], in0=fc_[:], scalar1=-1.0)
                nc.vector.tensor_max(out=Ac[:], in0=Ac[:], in1=fc_[:])
                return Ac

            D1_CH = ("dve", "dve", "pool")
            D1_ABS = ("act", "act", "act")

            def emit_front(tt_i):
                """Broadcast DMAs + frac chains + Sin-LUT tables for one tile."""
                pslc = slice(tt_i * NI, (tt_i + 1) * NI)
                b_psi = bc.tile([C1, NI], DT.float32, tag="b_psi")
                b_phi = bc.tile([D2, NI], DT.float32, tag="b_phi")
                b_tab = bc.tile([MT, NI], DT.float32, tag="b_tab")
                nc.sync.dma_start(b_psi[:], bpsi_d[0:1, pslc].to_broadcast((C1, NI)))
                nc.sync.dma_start(b_phi[:], bphi_d[0:1, pslc].to_broadcast((D2, NI)))
                nc.sync.dma_start(b_tab[:], btab_d[0:1, pslc].to_broadcast((MT, NI)))

                # D1 tables: bf16 sin/cos of b*r, b = c*99 + j
                sin1 = tb2.tile([C1, 3 * NI], DT.bfloat16, tag="sin1")
                cos1 = tb2.tile([C1, 3 * NI], DT.bfloat16, tag="cos1")
                D1_STT = ("dve", "dve", "dve")
                for c in range(3):
                    fc_, Ac = frac_chain(C1, b_psi, sc[:C1, c:c + 1], D1_CH[c], D1_STT[c])
                    nc.scalar.activation(out=sin1[:, c * NI:(c + 1) * NI], in_=fc_[:],
                                         func=AF.Sin, scale=TWO_PI_M)
                    u = abs_of(C1, fc_, Ac, D1_ABS[c])
                    nc.scalar.activation(out=cos1[:, c * NI:(c + 1) * NI], in_=u[:],
                                         func=AF.Sin, scale=-TWO_PI_M, bias=pi_half_t[:C1])

                # D2 tables
                sin2 = tb2.tile([D2, NI], DT.bfloat16, tag="sin2")
                cos2 = tb2.tile([D2, NI], DT.bfloat16, tag="cos2")
                f2_, A2 = frac_chain(D2, b_phi, sc[:D2, 3:4], "pool", "dve")
                nc.scalar.activation(out=sin2[:], in_=f2_[:], func=AF.Sin, scale=TWO_PI_M)
                u2 = abs_of(D2, f2_, A2, "act")
                nc.scalar.activation(out=cos2[:], in_=u2[:], func=AF.Sin,
                                     scale=-TWO_PI_M, bias=pi_half_t[:D2])

                # temporal cos table
                cost = tb2.tile([MT, NI], DT.bfloat16, tag="cost")
                # stt must be DVE: Pool has no scalar_tensor_tensor opcode
                f3_, A3 = frac_chain(MT, b_tab, sc[:MT, 4:5], "pool", "dve")
                u3 = abs_of(MT, f3_, A3, "act")
                nc.scalar.activation(out=cost[:], in_=u3[:], func=AF.Sin,
                                     scale=-TWO_PI_M, bias=pi_half_t[:MT])
                return sin1, cos1, sin2, cos2, cost

            def emit_tail(tt_i, tabs_):
                """Contraction matmuls + PSUM drain for one tile's tables."""
                sin1, cos1, sin2, cos2, cost = tabs_
                pslc = slice(tt_i * NI, (tt_i + 1) * NI)
                # matmuls per 512-column chunk; reduced rows accumulate in R
                # (temporal row lives at partition 32: matmul dest base
                # partition must be 0, 32, or 64)
                R = psr.tile([33, NI], DT.float32, tag="red")
                for q in range(NCHUNKS):
                    cs_ = slice(q * NCH, (q + 1) * NCH)
                    u_ps = ps.tile([D2, NCH], DT.float32, tag="u")
                    v_ps = ps.tile([D2, NCH], DT.float32, tag="v")
                    for c in range(3):
                        gcs = slice(c * NI + q * NCH, c * NI + (q + 1) * NCH)
                        nc.tensor.matmul(u_ps[:], wk[:, c * D2:(c + 1) * D2], cos1[:, gcs],
                                         start=(c == 0), stop=(c == 2))
                        nc.tensor.matmul(v_ps[:], wk[:, c * D2:(c + 1) * D2], sin1[:, gcs],
                                         start=(c == 0), stop=(c == 2))
                    t1m = sm.tile([D2, NCH], DT.bfloat16, tag="t1m")
                    t2m = sm.tile([D2, NCH], DT.bfloat16, tag="t2m")
                    nc.vector.tensor_mul(out=t1m[:], in0=sin2[:, cs_], in1=u_ps[:])
                    nc.vector.tensor_mul(out=t2m[:], in0=cos2[:, cs_], in1=v_ps[:])
                    nc.tensor.matmul(R[0:1, cs_], ones121[:], t1m[:], start=True, stop=False)
                    nc.tensor.matmul(R[0:1, cs_], ones121[:], t2m[:], start=False, stop=True)
                    nc.tensor.matmul(R[32:33, cs_], tkw[:], cost[:, cs_], start=True, stop=True)
                # both reduced rows PSUM->SBUF in one partition-parallel copy
                # (cost is column-bound, so copying 33 rows == copying 1)
                stg2 = sm.tile([33, NI], DT.float32, tag="stg2")
                nc.vector.tensor_copy(out=stg2[:], in_=R[:, :])
                # one partition-strided DMA grabs rows 0 and 32 together; with
                # the software-pipelined emission this sits BEHIND the next
                # tile's broadcasts in the SP queue, so it no longer stalls them
                nc.sync.dma_start(stg_both_d[0:2, pslc], stg2[0:33:32, :])

            # Software pipelining: tile i's matmul tail is emitted AFTER tile
            # i+1's frac stage, so in each engine's program order DVE starts
            # the next tile's chains instead of idling through the ACT->PE
            # latency of the current tile's tables.
            prev_tabs = None
            for tt_i in range(NTILES):
                cur_tabs = emit_front(tt_i)
                if prev_tabs is not None:
                    emit_tail(tt_i - 1, prev_tabs)
                prev_tabs = cur_tabs
            emit_tail(NTILES - 1, prev_tabs)

            # ---------------- tail: point-major combine ----------------
            spat_pm = pm.tile([128, 128], DT.float32)
            temp_pm = pm.tile([128, 128], DT.float32)
            nc.sync.dma_start(spat_pm[:], stg_both_d[0:1, :].rearrange("o (p f) -> (o p) f", p=128))
            nc.sync.dma_start(temp_pm[:], stg_both_d[1:2, :].rearrange("o (p f) -> (o p) f", p=128))
            spat2 = pm.tile([128, 128], DT.float32)
            nc.vector.tensor_mul(out=spat2[:], in0=spat_pm[:], in1=rdinv[:])
            temp2 = pm.tile([128, 128], DT.float32)
            nc.vector.tensor_mul(out=temp2[:], in0=temp_pm[:], in1=env_pm[:])
            nc.vector.tensor_mul(out=spat2[:], in0=spat2[:], in1=temp2[:])
            nc.vector.tensor_add(out=spat2[:], in0=spat2[:], in1=green[:])
            outt = pm.tile([128, 128], DT.float32)
            nc.vector.tensor_mul(out=outt[:], in0=spat2[:], in1=maskout[:])
            nc.sync.dma_start(out_d.rearrange("(p f) -> p f", p=128), outt[:])
    return nc


class SafeTileContext(tile.TileContext):
    """TileContext for a walrus build with tight per-instruction sync-wait
    limits (DMAs: 1, compute: 2). Excess waits are moved onto injected
    single-wait NOPs placed immediately before the instruction on the same
    engine, and the exit drain is split the same way."""

    _WAIT_LIMITS = {"InstDMACopy": 1, "InstDrain": 1, "InstMemSet": 1}
    _DEFAULT_WAIT_LIMIT = 1

    def schedule_and_allocate(self):
        ret = super().schedule_and_allocate()
        nc = self.nc
        eng_obj = {
            mybir.EngineType.PE: nc.tensor,
            mybir.EngineType.DVE: nc.vector,
            mybir.EngineType.Activation: nc.scalar,
            mybir.EngineType.Pool: nc.gpsimd,
            mybir.EngineType.SP: nc.sync,
        }
        # pass 1: collect instructions carrying too many waits
        fixes = []
        for bb in nc.main_func.blocks:
            insts = bb.instructions
            for i, ins in enumerate(insts):
                si = ins.sync_info
                waits = list(si.on_wait) if si and si.on_wait else []
                limit = self._WAIT_LIMITS.get(type(ins).__name__,
                                              self._DEFAULT_WAIT_LIMIT)
                if len(waits) > limit:
                    fixes.append((insts, i, ins, waits, limit))
        # pass 2: apply in reverse index order per list
        for insts, i, ins, waits, limit in sorted(fixes, key=lambda f: -f[1]):
            si = ins.sync_info
            ins.sync_info = mybir.SyncInfo(
                on_wait=waits[-limit:], on_update=list(si.on_update or []))
            at = i
            if (type(ins).__name__ == "InstMatmult" and i > 0
                    and type(insts[i - 1]).__name__ == "InstLdweights"):
                at = i - 1
            for j, w in enumerate(waits[:-limit]):
                nb = eng_obj[ins.engine].nop()
                nop_ins = nb.ins
                # relocate from wherever nop() appended it
                for bb2 in nc.main_func.blocks:
                    if bb2.instructions and bb2.instructions[-1] is nop_ins:
                        bb2.instructions.pop()
                        break
                nop_ins.sync_info = mybir.SyncInfo(on_wait=[w], on_update=[])
                insts.insert(at + j, nop_ins)
        return ret

    def _drain_and_barrier(self, tick_clock, wait_clock):
        nc = self.nc
        nop0 = nc.sync.nop()
        wait_clock.add_sem_waits(nop0.ins, tile.ScopedClock({None: tick_clock.global_clock}))
        waits = list(nop0.ins.sync_info.on_wait or []) if nop0.ins.sync_info else []
        if len(waits) > 1:
            upd = nop0.ins.sync_info.on_update or []
            nop0.ins.sync_info = mybir.SyncInfo(on_wait=[waits[0]], on_update=list(upd))
            for w in waits[1:]:
                nk = nc.sync.nop()
                nk.ins.sync_info = mybir.SyncInfo(on_wait=[w], on_update=[])
        nc.sync.drain()
        nc.all_engine_barrier()
        assert self.sems is not None
        popped = nc._tile_sem_poison_stack.pop()
        assert popped is self._sem_poison
        nc.clear_and_free_semaphores(list(self.sems.allocated().values()))
        nc.all_engine_barrier()


def _host_constants(spatial_kernel, temporal_kernel):
    k = np.asarray(spatial_kernel, dtype=f32)
    K = k.reshape(D2, D1)                       # K[a, b] = k[a*D1 + b]
    wk = np.empty((C1, 3 * D2), dtype=bf16)
    for c in range(3):
        wk[:, c * D2:(c + 1) * D2] = K[:, c * C1:(c + 1) * C1].T.astype(bf16)
    sc = np.zeros((128, 8), dtype=f32)
    p = np.arange(128, dtype=f32)
    sc[:, 0] = p
    sc[:, 1] = 99 + p
    sc[:, 2] = 198 + p
    sc[:, 3] = p
    freqs = ((np.arange(MT, dtype=f32) + f32(1.0)) * f32(0.1)).astype(f32)
    sc[:MT, 4] = (freqs * f32(INV2PI)).astype(f32)
    tkw = np.asarray(temporal_kernel, dtype=f32).reshape(MT, 1).astype(bf16)
    return wk, sc, tkw


class _Runner:
    """Compile once; keep the jitted shard_map executable, the device-resident
    output placeholder buffers, and (content-keyed) device-resident weights
    alive across kernel() calls so a warm call is a single PJRT dispatch."""

    def __init__(self, nc):
        import jax
        from jax.sharding import Mesh, PartitionSpec, NamedSharding
        from jax.experimental.shard_map import shard_map
        from concourse import bass2jax

        try:
            jax.config.update("jax_compilation_cache_dir",
                              "/tmp/nn_causal_jax_cache")
            jax.config.update("jax_persistent_cache_min_compile_time_secs", 0.0)
            jax.config.update("jax_persistent_cache_min_entry_size_bytes", -1)
        except Exception:
            pass
        bass2jax.install_neuronx_cc_hook()
        self._jax = jax
        partition_name = (nc.partition_id_tensor.name
                          if nc.partition_id_tensor else None)
        in_names, out_names, out_avals, zero_outs = [], [], [], []
        for alloc in nc.m.functions[0].allocations:
            if not isinstance(alloc, mybir.MemoryLocationSet):
                continue
            name = alloc.memorylocations[0].name
            if alloc.kind == "ExternalInput":
                if name != partition_name:
                    in_names.append(name)
            elif alloc.kind == "ExternalOutput":
                shape = tuple(alloc.tensor_shape)
                dtype = mybir.dt.np(alloc.dtype)
                out_names.append(name)
                out_avals.append(jax.core.ShapedArray(shape, dtype))
                zero_outs.append(np.zeros(shape, dtype))
        self.in_names = list(in_names)
        self.out_names = list(out_names)
        n_params = len(in_names)
        n_outs = len(out_avals)
        in_names_all = in_names + out_names
        if partition_name is not None:
            in_names_all.append(partition_name)

        def _body(*args):
            operands = list(args)
            if partition_name is not None:
                operands.append(bass2jax.partition_id_tensor())
            outs = bass2jax._bass_exec_p.bind(
                *operands,
                out_avals=tuple(out_avals),
                in_names=tuple(in_names_all),
                out_names=tuple(out_names),
                lowering_input_output_aliases=(),
                sim_require_finite=True,
                sim_require_nnan=True,
                nc=nc,
            )
            return tuple(outs)

        devices = jax.devices()[:N_CORES]
        assert len(devices) == N_CORES
        mesh = Mesh(np.asarray(devices), ("core",))
        self._sharding = NamedSharding(mesh, PartitionSpec("core"))
        self._fn = jax.jit(
            shard_map(_body, mesh=mesh,
                      in_specs=(PartitionSpec("core"),) * (n_params + n_outs),
                      out_specs=(PartitionSpec("core"),) * n_outs,
                      check_rep=False),
            keep_unused=True,
        )
        # The kernel fully writes its only output, so the zero placeholder
        # buffers are never read: keep them device-resident, undonated.
        self._zeros_dev = [
            jax.device_put(
                np.zeros((N_CORES * z.shape[0], *z.shape[1:]), z.dtype),
                self._sharding)
            for z in zero_outs
        ]
        self._const_key = None
        self._const_dev = None

    def __call__(self, ts_global, wk, sc, tkw):
        jax = self._jax
        ckey = (wk.tobytes(), sc.tobytes(), tkw.tobytes())
        if self._const_key != ckey:
            self._const_dev = {
                name: jax.device_put(
                    np.concatenate([arr] * N_CORES, axis=0), self._sharding)
                for name, arr in (("wk", wk), ("sc", sc), ("tkw", tkw))
            }
            self._const_key = ckey
        args = []
        for name in self.in_names:
            if name == "ts":
                args.append(ts_global)
            else:
                args.append(self._const_dev[name])
        out_arrs = self._fn(*args, *self._zeros_dev)
        return np.asarray(out_arrs[self.out_names.index("out")])


_RUNNER = None
_RUNNER_KEY = None


def _run_fallback(nc, ts, wk, sc, tkw):
    """Stock path: per-call run_bass_kernel_spmd (no executable caching)."""
    from concourse.bass_utils import run_bass_kernel_spmd
    in_maps = [{"ts": np.ascontiguousarray(ts[c * NPT:(c + 1) * NPT]),
                "wk": wk, "sc": sc, "tkw": tkw} for c in range(N_CORES)]
    res = run_bass_kernel_spmd(nc, in_maps, core_ids=list(range(N_CORES)))
    return np.concatenate([res.results[c]["out"] for c in range(N_CORES)])


def kernel(spacetime_coords, spatial_kernel, temporal_kernel,
           mass_parameter, coupling_strength):
    global _RUNNER, _RUNNER_KEY
    coords = np.asarray(spacetime_coords, dtype=np.float32)
    t = coords[:, 0]
    x = coords[:, 1]
    y = coords[:, 2]
    z = coords[:, 3]
    ts = np.empty((coords.shape[0], 2), dtype=np.float32)
    ts[:, 0] = t
    ts[:, 1] = (x * x + y * y) + z * z
    wk, sc, tkw = _host_constants(spatial_kernel, temporal_kernel)

    key = (float(np.float32(mass_parameter)), float(np.float32(coupling_strength)))
    if _RUNNER is None or _RUNNER_KEY != key:
        nc = _build_nc(*key)
        try:
            runner = _Runner(nc)
            runner(ts, wk, sc, tkw)   # warm the dispatch path during setup
        except Exception:
            return _run_fallback(nc, ts, wk, sc, tkw)
        _RUNNER = runner
        _RUNNER_KEY = key
    return _RUNNER(ts, wk, sc, tkw)


if __name__ == "__main__":
    rng = np.random.default_rng(0)
    ins = {
        "spacetime_coords": (rng.standard_normal((131072, 4)) * 2.0).astype(np.float32),
        "spatial_kernel": (rng.standard_normal(35937) * 0.1).astype(np.float32),
        "temporal_kernel": (rng.standard_normal(33) * 0.1).astype(np.float32),
        "mass_parameter": np.float32(1.0),
        "coupling_strength": np.float32(0.1),
    }
    out = kernel(**ins)
    print("out", out.shape, out.dtype, float(np.abs(out).max()))
    out2 = kernel(**ins)
    print("match:", np.array_equal(out, out2))


# revision 70
# speedup vs baseline: 1.1065x; 1.0117x over previous
"""Causal kernel (nn_CausalKernel) for 8x TRN2 NeuronCores.

Algorithm: sum_n k_n sin(n*r) decomposed via n = a*297 + b:
  sin(n r) = sin_a cos_b + cos_a sin_b with
  sin_b = sin(2pi frac(b * r/2pi)), sin_a = sin(2pi frac(a * 297r/2pi)).
Per-point trig tables are built mode-major ([modes, points]) with a
magic-number round chain (chain1: x*s+MAGIC, chain2: -MAGIC, stt: x*s-m)
producing the signed fraction f in [-0.5, 0.5], then the ScalarE Sin LUT
(valid range [-pi, pi]) maps sin(2pi f) directly and cos(2pi f) =
sin(pi/2 - 2pi|f|) with |f| from ScalarE Abs or a DVE neg+max pair. The
chain ops are distributed across DVE / Pool / ScalarE to balance engine
busy time; the 35937-mode contraction runs on TensorE in bf16.

Pure data parallel: 8 cores x 16384 points; weights replicated.

Host wrapper: the compiled executable (jit of shard_map over the 8-core
mesh) is built once and cached at module level; warm calls upload only the
[N, 2] (t, x^2+y^2+z^2) point data and download the [N] f32 output.
Weights and the (unused, non-donated) output placeholder buffers stay
resident on device between calls.
"""
import sys
sys.path.insert(0, "/opt/trn_rl_repo")

import numpy as np
import ml_dtypes

import concourse.bass as bass
import concourse.mybir as mybir
import concourse.tile as tile

f32 = np.float32
bf16 = ml_dtypes.bfloat16

N_CORES = 8
NPT = 16384            # points per core
NI = 2048              # points per point-tile
NTILES = NPT // NI     # 8
NCH = 512              # matmul moving-dim chunk (one PSUM bank)
NCHUNKS = NI // NCH    # 4

D1, D2 = 297, 121      # n = a*D1 + b
CH = (64, 117, 116)    # D1 contraction chunk rows; chunk 0 is 64 so the 33
B_OFF = (0, 64, 181)   # temporal rows ride at partitions 64..96 of its tile
MR = 97                # merged tile rows: 64 D1 + 33 temporal
CHMAX = 117
MT = 33                # temporal modes

MAGIC = float(f32(1.5 * 2 ** 23))
INV2PI = float(f32(1.0 / (2 * np.pi)))
TWO_PI_M = float(f32(6.2831845))   # < 2pi so LUT args stay inside [-pi, pi]
PI_HALF = float(f32(np.pi / 2))
DT = mybir.dt


def _build_nc(mass_parameter: float, coupling_strength: float):
    nc = bass.Bass(target_bir_lowering=False)
    AF = mybir.ActivationFunctionType
    OP = mybir.AluOpType

    ts_in = nc.dram_tensor("ts", [NPT, 2], DT.float32, kind="ExternalInput")
    wk_in = nc.dram_tensor("wk", [CHMAX, 3 * D2], DT.bfloat16, kind="ExternalInput")
    sc_in = nc.dram_tensor("sc", [128, 8], DT.float32, kind="ExternalInput")
    tkw_in = nc.dram_tensor("tkw", [MT, 1], DT.bfloat16, kind="ExternalInput")
    out_d = nc.dram_tensor("out", [NPT], DT.float32, kind="ExternalOutput")
    stg_both_d = nc.dram_tensor("stg_both", [2, NPT], DT.float32)
    bpsi_d = nc.dram_tensor("bpsi", [1, NPT], DT.float32)
    bphi_d = nc.dram_tensor("bphi", [1, NPT], DT.float32)
    btab_d = nc.dram_tensor("btab", [1, NPT], DT.float32)

    mp = float(f32(mass_parameter))
    cs = float(f32(coupling_strength))

    with SafeTileContext(nc) as tc:
        with (
            tc.tile_pool(name="const", bufs=1) as cpool,
            tc.tile_pool(name="pm", bufs=1) as pm,          # point-major persistents
            tc.tile_pool(name="bc", bufs=2) as bc,          # broadcast tiles
            tc.tile_pool(name="chain", bufs=3) as ch,       # frac chain scratch
            tc.tile_pool(name="small", bufs=2) as sm,       # t1m/t2m/stg2
            tc.tile_pool(name="tab", bufs=2) as tb,
            tc.tile_pool(name="tab2", bufs=2) as tb2,         # bf16 tables
            tc.tile_pool(name="ps", bufs=2, space="PSUM") as ps,
            tc.tile_pool(name="psr", bufs=1, space="PSUM") as psr,
        ):
            # ---------------- constants ----------------
            sc0 = cpool.tile([128, 8], DT.float32)
            nc.sync.dma_start(sc0[:], sc_in[:])
            sc = cpool.tile([128, 8], DT.float32)
            nc.vector.tensor_copy(out=sc[:], in_=sc0[:])    # absorb DMA sem on DVE
            wk0 = cpool.tile([CHMAX, 3 * D2], DT.bfloat16)
            nc.sync.dma_start(wk0[:], wk_in[:])
            wk = cpool.tile([CHMAX, 3 * D2], DT.bfloat16)
            nc.vector.tensor_copy(out=wk[:], in_=wk0[:])

            tkw0 = cpool.tile([MT, 1], DT.bfloat16)
            nc.sync.dma_start(tkw0[:], tkw_in[:])
            # temporal weights live at partitions 64..96 so the lhsT base
            # matches the merged cos table's temporal rows (rhs base 64)
            tkw = cpool.tile([MR, 1], DT.bfloat16)
            nc.vector.tensor_copy(out=tkw[64:MR, :], in_=tkw0[:])

            ones121 = cpool.tile([D2, 1], DT.bfloat16)
            nc.vector.memset(ones121[:], 1.0)
            pi_half_t = cpool.tile([128, 1], DT.float32)
            nc.vector.memset(pi_half_t[:], PI_HALF)
            magic_t = cpool.tile([128, 1], DT.float32)
            nc.vector.memset(magic_t[:], MAGIC)
            nmagic_t = cpool.tile([128, 1], DT.float32)
            nc.vector.memset(nmagic_t[:], -MAGIC)

            # ---------------- stage 0: point-major precompute ----------------
            crd = pm.tile([128, 256], DT.float32)
            nc.sync.dma_start(crd[:], ts_in.rearrange("(p f) c -> p (f c)", p=128))
            crd2 = crd[:].rearrange("p (f c) -> p f c", c=2)

            t_pm = pm.tile([128, 128], DT.float32)
            nc.vector.tensor_copy(out=t_pm[:], in_=crd2[:, :, 0])
            sdsq = pm.tile([128, 128], DT.float32)
            nc.vector.tensor_copy(out=sdsq[:], in_=crd2[:, :, 1])
            r2e = pm.tile([128, 128], DT.float32)
            nc.vector.tensor_scalar_add(out=r2e[:], in0=sdsq[:], scalar1=float(f32(1e-12)))

            # r = sqrt(r2e) with two Newton refinements (HW sqrt LUT is loose)
            r_pm = pm.tile([128, 128], DT.float32)
            nc.scalar.activation(out=r_pm[:], in_=r2e[:], func=AF.Sqrt)
            tmpa = pm.tile([128, 128], DT.float32, tag="w1")
            tmpb = pm.tile([128, 128], DT.float32, tag="w2")
            for _ in range(2):
                nc.vector.reciprocal(out=tmpa[:], in_=r_pm[:])
                nc.vector.tensor_mul(out=tmpb[:], in0=r2e[:], in1=tmpa[:])
                nc.vector.tensor_add(out=tmpb[:], in0=tmpb[:], in1=r_pm[:])
                nc.vector.tensor_scalar_mul(out=r_pm[:], in0=tmpb[:], scalar1=0.5)

            # psi1 = frac(r/2pi), signed
            A0 = pm.tile([128, 128], DT.float32)
            m0 = pm.tile([128, 128], DT.float32)
            psi1 = pm.tile([128, 128], DT.float32)
            nc.vector.tensor_scalar(out=A0[:], in0=r_pm[:], scalar1=INV2PI,
                                    scalar2=MAGIC, op0=OP.mult, op1=OP.add)
            nc.vector.tensor_scalar_add(out=m0[:], in0=A0[:], scalar1=-MAGIC)
            nc.vector.scalar_tensor_tensor(out=psi1[:], in0=r_pm[:], scalar=INV2PI,
                                           in1=m0[:], op0=OP.mult, op1=OP.subtract)

            # phi1 = frac(D1 * r / 2pi) via 12-bit split of r (accuracy for a<=120 amplification)
            SC12 = float(f32(2.0 ** 12))
            c2_64 = np.float64(D1) / (2 * np.pi)
            c2h = float(f32(np.trunc(c2_64 * 2 ** 12) / 2 ** 12))
            c2l = float(f32(c2_64 - np.float64(f32(c2h))))
            c2f = float(f32(c2_64))
            rh = pm.tile([128, 128], DT.float32)
            rl = pm.tile([128, 128], DT.float32)
            nc.vector.tensor_scalar(out=A0[:], in0=r_pm[:], scalar1=SC12,
                                    scalar2=MAGIC, op0=OP.mult, op1=OP.add)
            nc.vector.tensor_scalar_add(out=m0[:], in0=A0[:], scalar1=-MAGIC)
            nc.vector.tensor_scalar_mul(out=rh[:], in0=m0[:], scalar1=float(f32(2.0 ** -12)))
            nc.vector.tensor_sub(out=rl[:], in0=r_pm[:], in1=rh[:])
            # t1 = rh*c2h (exact); f1 = frac(t1)
            t1t = pm.tile([128, 128], DT.float32, tag="w3")
            nc.vector.tensor_scalar(out=A0[:], in0=rh[:], scalar1=c2h,
                                    scalar2=MAGIC, op0=OP.mult, op1=OP.add)
            nc.vector.tensor_scalar_add(out=m0[:], in0=A0[:], scalar1=-MAGIC)
            nc.vector.scalar_tensor_tensor(out=t1t[:], in0=rh[:], scalar=c2h,
                                           in1=m0[:], op0=OP.mult, op1=OP.subtract)
            # rest = rh*c2l + rl*c2 ; ph = f1 + rest ; phi1 = frac(ph)
            nc.vector.tensor_scalar_mul(out=tmpa[:], in0=rl[:], scalar1=c2f)
            nc.vector.scalar_tensor_tensor(out=tmpb[:], in0=rh[:], scalar=c2l,
                                           in1=tmpa[:], op0=OP.mult, op1=OP.add)
            ph_t = pm.tile([128, 128], DT.float32)
            nc.vector.tensor_add(out=ph_t[:], in0=t1t[:], in1=tmpb[:])
            phi1 = pm.tile([128, 128], DT.float32)
            nc.vector.tensor_scalar(out=A0[:], in0=ph_t[:], scalar1=1.0,
                                    scalar2=MAGIC, op0=OP.mult, op1=OP.add)
            nc.vector.tensor_scalar_add(out=m0[:], in0=A0[:], scalar1=-MAGIC)
            nc.vector.tensor_sub(out=phi1[:], in0=ph_t[:], in1=m0[:])

            # |t|, envelope, green, masks, 1/(r+1e-6)
            tabs = pm.tile([128, 128], DT.float32)
            nc.vector.tensor_scalar_mul(out=tabs[:], in0=t_pm[:], scalar1=-1.0)
            nc.vector.tensor_max(out=tabs[:], in0=tabs[:], in1=t_pm[:])

            # bases to DRAM for broadcast-DMA sourcing — stored as soon as they
            # are ready so tile 0's broadcasts overlap the mask/green tail below
            nc.sync.dma_start(bpsi_d[:].rearrange("o (p f) -> (o p) f", p=128), psi1[:])
            nc.sync.dma_start(bphi_d[:].rearrange("o (p f) -> (o p) f", p=128), phi1[:])
            nc.sync.dma_start(btab_d[:].rearrange("o (p f) -> (o p) f", p=128), tabs[:])

            env_pm = pm.tile([128, 128], DT.float32)
            nc.scalar.activation(out=env_pm[:], in_=tabs[:], func=AF.Exp,
                                 scale=float(f32(-0.1)))
            expg = pm.tile([128, 128], DT.float32)
            nc.scalar.activation(out=expg[:], in_=r_pm[:], func=AF.Exp, scale=-mp)
            rinv = pm.tile([128, 128], DT.float32)
            nc.vector.reciprocal(out=rinv[:], in_=r_pm[:])
            green = pm.tile([128, 128], DT.float32)
            nc.vector.tensor_mul(out=green[:], in0=expg[:], in1=rinv[:])
            nc.vector.tensor_scalar_mul(out=green[:], in0=green[:], scalar1=cs)
            rden = pm.tile([128, 128], DT.float32)
            nc.vector.tensor_scalar_add(out=rden[:], in0=r_pm[:], scalar1=float(f32(1e-6)))
            rdinv = pm.tile([128, 128], DT.float32)
            nc.vector.reciprocal(out=rdinv[:], in_=rden[:])

            tsq = pm.tile([128, 128], DT.float32)
            nc.vector.tensor_mul(out=tsq[:], in0=t_pm[:], in1=t_pm[:])
            interval = pm.tile([128, 128], DT.float32)
            nc.vector.tensor_sub(out=interval[:], in0=tsq[:], in1=sdsq[:])
            mg1 = pm.tile([128, 128], DT.float32, tag="w4")
            mg2 = pm.tile([128, 128], DT.float32, tag="w5")
            nc.vector.tensor_scalar(out=mg1[:], in0=interval[:], scalar1=0.0,
                                    scalar2=None, op0=OP.is_gt)
            nc.vector.tensor_scalar(out=mg2[:], in0=t_pm[:], scalar1=0.0,
                                    scalar2=None, op0=OP.is_gt)
            nc.vector.tensor_mul(out=mg1[:], in0=mg1[:], in1=mg2[:])
            nc.vector.tensor_mul(out=green[:], in0=green[:], in1=mg1[:])
            mo1 = pm.tile([128, 128], DT.float32, tag="w4")
            mo2 = pm.tile([128, 128], DT.float32, tag="w5")
            nc.vector.tensor_scalar(out=mo1[:], in0=interval[:], scalar1=0.0,
                                    scalar2=None, op0=OP.is_ge)
            nc.vector.tensor_scalar(out=mo2[:], in0=t_pm[:], scalar1=0.0,
                                    scalar2=None, op0=OP.is_ge)
            maskout = pm.tile([128, 128], DT.float32)
            nc.vector.tensor_mul(out=maskout[:], in0=mo1[:], in1=mo2[:])

            # ---------------- per point-tile mode-major pipeline ----------------
            # signed frac f = base*scal - round(base*scal), chain engines
            # picked per group to balance DVE / Pool / ACT busy time.
            def frac_chain(P, base, scal, ch_eng, stt_eng):
                Ac = ch.tile([P, NI], DT.float32, tag="Ac")
                fc_ = ch.tile([P, NI], DT.float32, tag="fc")
                bslc = base[0:P, :]
                if ch_eng == "act":
                    nc.scalar.activation(out=Ac[:], in_=bslc, func=AF.Identity,
                                         bias=magic_t[:P], scale=scal)
                    nc.scalar.activation(out=Ac[:], in_=Ac[:], func=AF.Identity,
                                         bias=nmagic_t[:P], scale=1.0)
                else:
                    eng = nc.vector if ch_eng == "dve" else nc.gpsimd
                    eng.tensor_scalar(out=Ac[:], in0=bslc, scalar1=scal,
                                      scalar2=MAGIC, op0=OP.mult, op1=OP.add)
                    eng.tensor_scalar_add(out=Ac[:], in0=Ac[:], scalar1=-MAGIC)
                seng = nc.vector if stt_eng == "dve" else nc.gpsimd
                seng.scalar_tensor_tensor(out=fc_[:], in0=bslc, scalar=scal,
                                          in1=Ac[:], op0=OP.mult, op1=OP.subtract)
                return fc_, Ac

            def abs_of(P, fc_, Ac, eng):
                # writes |f| into Ac in place (its chain value is dead)
                if eng == "act":
                    nc.scalar.activation(out=Ac[:], in_=fc_[:], func=AF.Abs)
                    return Ac
                nc.vector.tensor_scalar_mul(out=Ac[:], in0=fc_[:], scalar1=-1.0)
                nc.vector.tensor_max(out=Ac[:], in0=Ac[:], in1=fc_[:])
                return Ac

            def emit_front(tt_i):
                """Broadcast DMAs + frac chains + Sin-LUT tables for one tile.

                Group g0 is a merged [97, NI] tile: partitions 0..63 hold D1
                b=0..63 (matmul rhs base 0) and partitions 64..96 hold the 33
                temporal rows (rhs base 64) — the temporal abs/cos ride the
                D1 chunk's column-bound ops for free."""
                pslc = slice(tt_i * NI, (tt_i + 1) * NI)
                bM = bc.tile([MR, NI], DT.float32, tag="bM")
                b_psi = bc.tile([CHMAX, NI], DT.float32, tag="b_psi")
                b_phi = bc.tile([D2, NI], DT.float32, tag="b_phi")
                nc.sync.dma_start(bM[0:64, :], bpsi_d[0:1, pslc].to_broadcast((64, NI)))
                nc.sync.dma_start(bM[64:MR, :], btab_d[0:1, pslc].to_broadcast((MT, NI)))
                nc.sync.dma_start(b_psi[:], bpsi_d[0:1, pslc].to_broadcast((CHMAX, NI)))
                nc.sync.dma_start(b_phi[:], bphi_d[0:1, pslc].to_broadcast((D2, NI)))

                # g0 (merged): sin/cos of b*r rows 0..63, temporal cos rows 64..96
                sinM = tb2.tile([MR, NI], DT.bfloat16, tag="sinM")
                cosM = tb2.tile([MR, NI], DT.bfloat16, tag="cosM")
                fM, AM = frac_chain(MR, bM, sc[:MR, 0:1], "dve", "dve")
                nc.scalar.activation(out=sinM[:], in_=fM[:], func=AF.Sin, scale=TWO_PI_M)
                uM = abs_of(MR, fM, AM, "act")
                nc.scalar.activation(out=cosM[:], in_=uM[:], func=AF.Sin,
                                     scale=-TWO_PI_M, bias=pi_half_t[:MR])

                # g1/g2: D1 b = 64..180 / 181..296
                sin1b = tb2.tile([CH[1], NI], DT.bfloat16, tag="sin1b")
                cos1b = tb2.tile([CH[1], NI], DT.bfloat16, tag="cos1b")
                f1_, A1 = frac_chain(CH[1], b_psi, sc[:CH[1], 1:2], "dve", "dve")
                nc.scalar.activation(out=sin1b[:], in_=f1_[:], func=AF.Sin, scale=TWO_PI_M)
                u1 = abs_of(CH[1], f1_, A1, "act")
                nc.scalar.activation(out=cos1b[:], in_=u1[:], func=AF.Sin,
                                     scale=-TWO_PI_M, bias=pi_half_t[:CH[1]])

                sin1c = tb2.tile([CH[2], NI], DT.bfloat16, tag="sin1c")
                cos1c = tb2.tile([CH[2], NI], DT.bfloat16, tag="cos1c")
                f2c, A2c = frac_chain(CH[2], b_psi, sc[:CH[2], 2:3], "pool", "dve")
                nc.scalar.activation(out=sin1c[:], in_=f2c[:], func=AF.Sin, scale=TWO_PI_M)
                u2c = abs_of(CH[2], f2c, A2c, "act")
                nc.scalar.activation(out=cos1c[:], in_=u2c[:], func=AF.Sin,
                                     scale=-TWO_PI_M, bias=pi_half_t[:CH[2]])

                # D2 tables
                sin2 = tb2.tile([D2, NI], DT.bfloat16, tag="sin2")
                cos2 = tb2.tile([D2, NI], DT.bfloat16, tag="cos2")
                f2_, A2 = frac_chain(D2, b_phi, sc[:D2, 3:4], "pool", "dve")
                nc.scalar.activation(out=sin2[:], in_=f2_[:], func=AF.Sin, scale=TWO_PI_M)
                u2 = abs_of(D2, f2_, A2, "act")
                nc.scalar.activation(out=cos2[:], in_=u2[:], func=AF.Sin,
                                     scale=-TWO_PI_M, bias=pi_half_t[:D2])
                return sinM, cosM, sin1b, cos1b, sin1c, cos1c, sin2, cos2

            def emit_tail(tt_i, tabs_):
                """Contraction matmuls + PSUM drain for one tile's tables."""
                sinM, cosM, sin1b, cos1b, sin1c, cos1c, sin2, cos2 = tabs_
                pslc = slice(tt_i * NI, (tt_i + 1) * NI)
                sins = (sinM, sin1b, sin1c)
                coss = (cosM, cos1b, cos1c)
                # matmuls per 512-column chunk; reduced rows accumulate in R
                # (temporal row lives at partition 32: matmul dest base
                # partition must be 0, 32, or 64)
                R = psr.tile([33, NI], DT.float32, tag="red")
                for q in range(NCHUNKS):
                    cs_ = slice(q * NCH, (q + 1) * NCH)
                    u_ps = ps.tile([D2, NCH], DT.float32, tag="u")
                    v_ps = ps.tile([D2, NCH], DT.float32, tag="v")
                    for c in range(3):
                        nc.tensor.matmul(u_ps[:], wk[0:CH[c], c * D2:(c + 1) * D2],
                                         coss[c][0:CH[c], cs_],
                                         start=(c == 0), stop=(c == 2))
                        nc.tensor.matmul(v_ps[:], wk[0:CH[c], c * D2:(c + 1) * D2],
                                         sins[c][0:CH[c], cs_],
                                         start=(c == 0), stop=(c == 2))
                    t1m = sm.tile([D2, NCH], DT.bfloat16, tag="t1m")
                    t2m = sm.tile([D2, NCH], DT.bfloat16, tag="t2m")
                    nc.vector.tensor_mul(out=t1m[:], in0=sin2[:, cs_], in1=u_ps[:])
                    nc.vector.tensor_mul(out=t2m[:], in0=cos2[:, cs_], in1=v_ps[:])
                    nc.tensor.matmul(R[0:1, cs_], ones121[:], t1m[:], start=True, stop=False)
                    nc.tensor.matmul(R[0:1, cs_], ones121[:], t2m[:], start=False, stop=True)
                    nc.tensor.matmul(R[32:33, cs_], tkw[64:MR, :], cosM[64:MR, cs_],
                                     start=True, stop=True)
                # both reduced rows PSUM->SBUF in one partition-parallel copy
                # (cost is column-bound, so copying 33 rows == copying 1)
                stg2 = sm.tile([33, NI], DT.float32, tag="stg2")
                nc.vector.tensor_copy(out=stg2[:], in_=R[:, :])
                # one partition-strided DMA grabs rows 0 and 32 together; with
                # the software-pipelined emission this sits BEHIND the next
                # tile's broadcasts in the SP queue, so it no longer stalls them
                nc.sync.dma_start(stg_both_d[0:2, pslc], stg2[0:33:32, :])

            # Software pipelining: tile i's matmul tail is emitted AFTER tile
            # i+1's frac stage, so in each engine's program order DVE starts
            # the next tile's chains instead of idling through the ACT->PE
            # latency of the current tile's tables.
            prev_tabs = None
            for tt_i in range(NTILES):
                cur_tabs = emit_front(tt_i)
                if prev_tabs is not None:
                    emit_tail(tt_i - 1, prev_tabs)
                prev_tabs = cur_tabs
            emit_tail(NTILES - 1, prev_tabs)

            # ---------------- tail: point-major combine ----------------
            spat_pm = pm.tile([128, 128], DT.float32)
            temp_pm = pm.tile([128, 128], DT.float32)
            nc.sync.dma_start(spat_pm[:], stg_both_d[0:1, :].rearrange("o (p f) -> (o p) f", p=128))
            nc.sync.dma_start(temp_pm[:], stg_both_d[1:2, :].rearrange("o (p f) -> (o p) f", p=128))
            spat2 = pm.tile([128, 128], DT.float32)
            nc.vector.tensor_mul(out=spat2[:], in0=spat_pm[:], in1=rdinv[:])
            temp2 = pm.tile([128, 128], DT.float32)
            nc.vector.tensor_mul(out=temp2[:], in0=temp_pm[:], in1=env_pm[:])
            nc.vector.tensor_mul(out=spat2[:], in0=spat2[:], in1=temp2[:])
            nc.vector.tensor_add(out=spat2[:], in0=spat2[:], in1=green[:])
            outt = pm.tile([128, 128], DT.float32)
            nc.vector.tensor_mul(out=outt[:], in0=spat2[:], in1=maskout[:])
            nc.sync.dma_start(out_d.rearrange("(p f) -> p f", p=128), outt[:])
    return nc


class SafeTileContext(tile.TileContext):
    """TileContext for a walrus build with tight per-instruction sync-wait
    limits (DMAs: 1, compute: 2). Excess waits are moved onto injected
    single-wait NOPs placed immediately before the instruction on the same
    engine, and the exit drain is split the same way."""

    _WAIT_LIMITS = {"InstDMACopy": 1, "InstDrain": 1, "InstMemSet": 1}
    _DEFAULT_WAIT_LIMIT = 1

    def schedule_and_allocate(self):
        ret = super().schedule_and_allocate()
        nc = self.nc
        eng_obj = {
            mybir.EngineType.PE: nc.tensor,
            mybir.EngineType.DVE: nc.vector,
            mybir.EngineType.Activation: nc.scalar,
            mybir.EngineType.Pool: nc.gpsimd,
            mybir.EngineType.SP: nc.sync,
        }
        # pass 1: collect instructions carrying too many waits
        fixes = []
        for bb in nc.main_func.blocks:
            insts = bb.instructions
            for i, ins in enumerate(insts):
                si = ins.sync_info
                waits = list(si.on_wait) if si and si.on_wait else []
                limit = self._WAIT_LIMITS.get(type(ins).__name__,
                                              self._DEFAULT_WAIT_LIMIT)
                if len(waits) > limit:
                    fixes.append((insts, i, ins, waits, limit))
        # pass 2: apply in reverse index order per list
        for insts, i, ins, waits, limit in sorted(fixes, key=lambda f: -f[1]):
            si = ins.sync_info
            ins.sync_info = mybir.SyncInfo(
                on_wait=waits[-limit:], on_update=list(si.on_update or []))
            at = i
            if (type(ins).__name__ == "InstMatmult" and i > 0
                    and type(insts[i - 1]).__name__ == "InstLdweights"):
                at = i - 1
            for j, w in enumerate(waits[:-limit]):
                nb = eng_obj[ins.engine].nop()
                nop_ins = nb.ins
                # relocate from wherever nop() appended it
                for bb2 in nc.main_func.blocks:
                    if bb2.instructions and bb2.instructions[-1] is nop_ins:
                        bb2.instructions.pop()
                        break
                nop_ins.sync_info = mybir.SyncInfo(on_wait=[w], on_update=[])
                insts.insert(at + j, nop_ins)
        return ret

    def _drain_and_barrier(self, tick_clock, wait_clock):
        nc = self.nc
        nop0 = nc.sync.nop()
        wait_clock.add_sem_waits(nop0.ins, tile.ScopedClock({None: tick_clock.global_clock}))
        waits = list(nop0.ins.sync_info.on_wait or []) if nop0.ins.sync_info else []
        if len(waits) > 1:
            upd = nop0.ins.sync_info.on_update or []
            nop0.ins.sync_info = mybir.SyncInfo(on_wait=[waits[0]], on_update=list(upd))
            for w in waits[1:]:
                nk = nc.sync.nop()
                nk.ins.sync_info = mybir.SyncInfo(on_wait=[w], on_update=[])
        nc.sync.drain()
        nc.all_engine_barrier()
        assert self.sems is not None
        popped = nc._tile_sem_poison_stack.pop()
        assert popped is self._sem_poison
        nc.clear_and_free_semaphores(list(self.sems.allocated().values()))
        nc.all_engine_barrier()


def _host_constants(spatial_kernel, temporal_kernel):
    k = np.asarray(spatial_kernel, dtype=f32)
    K = k.reshape(D2, D1)                       # K[a, b] = k[a*D1 + b]
    wk = np.zeros((CHMAX, 3 * D2), dtype=bf16)
    for c in range(3):
        wk[:CH[c], c * D2:(c + 1) * D2] = (
            K[:, B_OFF[c]:B_OFF[c] + CH[c]].T.astype(bf16))
    sc = np.zeros((128, 8), dtype=f32)
    freqs = ((np.arange(MT, dtype=f32) + f32(1.0)) * f32(0.1)).astype(f32)
    # col 0: merged group — b=0..63 then the 33 temporal scales
    sc[:64, 0] = np.arange(64, dtype=f32)
    sc[64:MR, 0] = (freqs * f32(INV2PI)).astype(f32)
    sc[:CH[1], 1] = B_OFF[1] + np.arange(CH[1], dtype=f32)
    sc[:CH[2], 2] = B_OFF[2] + np.arange(CH[2], dtype=f32)
    sc[:D2, 3] = np.arange(D2, dtype=f32)
    tkw = np.asarray(temporal_kernel, dtype=f32).reshape(MT, 1).astype(bf16)
    return wk, sc, tkw


class _Runner:
    """Compile once; keep the jitted shard_map executable, the device-resident
    output placeholder buffers, and (content-keyed) device-resident weights
    alive across kernel() calls so a warm call is a single PJRT dispatch."""

    def __init__(self, nc):
        import jax
        from jax.sharding import Mesh, PartitionSpec, NamedSharding
        from jax.experimental.shard_map import shard_map
        from concourse import bass2jax

        try:
            jax.config.update("jax_compilation_cache_dir",
                              "/tmp/nn_causal_jax_cache")
            jax.config.update("jax_persistent_cache_min_compile_time_secs", 0.0)
            jax.config.update("jax_persistent_cache_min_entry_size_bytes", -1)
        except Exception:
            pass
        bass2jax.install_neuronx_cc_hook()
        self._jax = jax
        partition_name = (nc.partition_id_tensor.name
                          if nc.partition_id_tensor else None)
        in_names, out_names, out_avals, zero_outs = [], [], [], []
        for alloc in nc.m.functions[0].allocations:
            if not isinstance(alloc, mybir.MemoryLocationSet):
                continue
            name = alloc.memorylocations[0].name
            if alloc.kind == "ExternalInput":
                if name != partition_name:
                    in_names.append(name)
            elif alloc.kind == "ExternalOutput":
                shape = tuple(alloc.tensor_shape)
                dtype = mybir.dt.np(alloc.dtype)
                out_names.append(name)
                out_avals.append(jax.core.ShapedArray(shape, dtype))
                zero_outs.append(np.zeros(shape, dtype))
        self.in_names = list(in_names)
        self.out_names = list(out_names)
        n_params = len(in_names)
        n_outs = len(out_avals)
        in_names_all = in_names + out_names
        if partition_name is not None:
            in_names_all.append(partition_name)

        def _body(*args):
            operands = list(args)
            if partition_name is not None:
                operands.append(bass2jax.partition_id_tensor())
            outs = bass2jax._bass_exec_p.bind(
                *operands,
                out_avals=tuple(out_avals),
                in_names=tuple(in_names_all),
                out_names=tuple(out_names),
                lowering_input_output_aliases=(),
                sim_require_finite=True,
                sim_require_nnan=True,
                nc=nc,
            )
            return tuple(outs)

        devices = jax.devices()[:N_CORES]
        assert len(devices) == N_CORES
        mesh = Mesh(np.asarray(devices), ("core",))
        self._sharding = NamedSharding(mesh, PartitionSpec("core"))
        self._fn = jax.jit(
            shard_map(_body, mesh=mesh,
                      in_specs=(PartitionSpec("core"),) * (n_params + n_outs),
                      out_specs=(PartitionSpec("core"),) * n_outs,
                      check_rep=False),
            keep_unused=True,
        )
        # The kernel fully writes its only output, so the zero placeholder
        # buffers are never read: keep them device-resident, undonated.
        self._zeros_dev = [
            jax.device_put(
                np.zeros((N_CORES * z.shape[0], *z.shape[1:]), z.dtype),
                self._sharding)
            for z in zero_outs
        ]
        self._const_key = None
        self._const_dev = None

    def __call__(self, ts_global, wk, sc, tkw):
        jax = self._jax
        ckey = (wk.tobytes(), sc.tobytes(), tkw.tobytes())
        if self._const_key != ckey:
            self._const_dev = {
                name: jax.device_put(
                    np.concatenate([arr] * N_CORES, axis=0), self._sharding)
                for name, arr in (("wk", wk), ("sc", sc), ("tkw", tkw))
            }
            self._const_key = ckey
        args = []
        for name in self.in_names:
            if name == "ts":
                args.append(ts_global)
            else:
                args.append(self._const_dev[name])
        out_arrs = self._fn(*args, *self._zeros_dev)
        return np.asarray(out_arrs[self.out_names.index("out")])


_RUNNER = None
_RUNNER_KEY = None


def _run_fallback(nc, ts, wk, sc, tkw):
    """Stock path: per-call run_bass_kernel_spmd (no executable caching)."""
    from concourse.bass_utils import run_bass_kernel_spmd
    in_maps = [{"ts": np.ascontiguousarray(ts[c * NPT:(c + 1) * NPT]),
                "wk": wk, "sc": sc, "tkw": tkw} for c in range(N_CORES)]
    res = run_bass_kernel_spmd(nc, in_maps, core_ids=list(range(N_CORES)))
    return np.concatenate([res.results[c]["out"] for c in range(N_CORES)])


def kernel(spacetime_coords, spatial_kernel, temporal_kernel,
           mass_parameter, coupling_strength):
    global _RUNNER, _RUNNER_KEY
    coords = np.asarray(spacetime_coords, dtype=np.float32)
    t = coords[:, 0]
    x = coords[:, 1]
    y = coords[:, 2]
    z = coords[:, 3]
    ts = np.empty((coords.shape[0], 2), dtype=np.float32)
    ts[:, 0] = t
    ts[:, 1] = (x * x + y * y) + z * z
    wk, sc, tkw = _host_constants(spatial_kernel, temporal_kernel)

    key = (float(np.float32(mass_parameter)), float(np.float32(coupling_strength)))
    if _RUNNER is None or _RUNNER_KEY != key:
        nc = _build_nc(*key)
        try:
            runner = _Runner(nc)
            runner(ts, wk, sc, tkw)   # warm the dispatch path during setup
        except Exception:
            return _run_fallback(nc, ts, wk, sc, tkw)
        _RUNNER = runner
        _RUNNER_KEY = key
    return _RUNNER(ts, wk, sc, tkw)


if __name__ == "__main__":
    rng = np.random.default_rng(0)
    ins = {
        "spacetime_coords": (rng.standard_normal((131072, 4)) * 2.0).astype(np.float32),
        "spatial_kernel": (rng.standard_normal(35937) * 0.1).astype(np.float32),
        "temporal_kernel": (rng.standard_normal(33) * 0.1).astype(np.float32),
        "mass_parameter": np.float32(1.0),
        "coupling_strength": np.float32(0.1),
    }
    out = kernel(**ins)
    print("out", out.shape, out.dtype, float(np.abs(out).max()))
    out2 = kernel(**ins)
    print("match:", np.array_equal(out, out2))
